# revision 1
# baseline (speedup 1.0000x reference)
"""MoE (E=4 experts, top-2 routing) forward pass on 8 Trainium2 NeuronCores.

Strategy: data-parallel over tokens. Full input x is [8, 2048, 1024]; core i
processes batch row i (2048 tokens). Expert weights are replicated to every
core. All experts are computed densely per token (E=4, top-2 -> 2x extra
matmul work, but no data-dependent routing), then combined with the top-2
softmax weights.

Per-core pipeline (T=2048 tokens, D=1024, E=4):
  prologue: PE-transpose x into x^T (bf16 for matmul lhsT + fp32 for gating),
            fp32 gate matmuls -> top-2 softmax weights (fp32: routing needs
            fp32 precision; min top2/top3 score gap on real data is ~2e-5).
  per (expert, token-tile):
    z    = x @ W1 + b1        PE, bf16 operands, fp32 PSUM (b1 via K=1 matmul)
    LN1 stats                 DVE bn_stats/bn_aggr reading PSUM
    n1   = (z - m)*rstd       ACT (per-partition scale/bias), PSUM -> SBUF
    n1  *= g1                 DVE tensor_tensor (in-place)
    n1  += be1                GPSIMD tensor_tensor (in-place)
    u    = relu(n1)           ACT, bf16 out
    u^T                       PE transpose (8x 128x128 bf16 blocks)
    z2   = u @ W2 + b2        PE bf16
    LN2 stats                 DVE
    n2   = (z2 - m2)*rstd2*w_e  ACT (w_e folded into the scale)
    n2  *= g2                 DVE (in-place)
    acc += n2                 GPSIMD (e=0 does acc = n2 + x residual)
  finalize per token-tile:
    C    = w @ be2            PE (K=4 matmul over experts)
    out  = C + acc            DVE, then DMA out
"""

import threading

import numpy as np

import concourse.bass as bass
import concourse.mybir as mybir
import concourse.tile as tile
from concourse import bacc
from concourse.bass import ds, ts
from concourse.masks import make_identity

F32 = mybir.dt.float32
BF16 = mybir.dt.bfloat16
AF = mybir.ActivationFunctionType
ALU = mybir.AluOpType
AX = mybir.AxisListType

P = 128
D = 1024
E = 4
KC = D // P  # contraction chunks per matmul
NCH = D // 512  # psum column chunks
LN_EPS = 1e-5
N_CORES = 8


def _row1(ap):
    """Lift an AP to have a leading length-1 (partition) dim."""
    return bass.AP(tensor=ap.tensor, offset=ap.offset, ap=[[0, 1]] + list(ap.ap))


def _bcast_rows(ap_row, p=P):
    """Broadcast a [1, N]-ish DRAM AP across p partitions (step-0 partition dim)."""
    inner = [list(d) for d in ap_row.ap if d[1] != 1]
    return bass.AP(tensor=ap_row.tensor, offset=ap_row.offset, ap=[[0, p]] + inner)


def build_moe_nc(T=2048, num_devices=N_CORES):
    TT = T // P
    nc = bacc.Bacc(
        "TRN2", target_bir_lowering=False, debug=False, num_devices=num_devices
    )

    x_d = nc.dram_tensor("x", [T, D], F32, kind="ExternalInput")
    gw_d = nc.dram_tensor("gate_W", [D, E], F32, kind="ExternalInput")
    gb_d = nc.dram_tensor("gate_b", [E], F32, kind="ExternalInput")
    w1_d = nc.dram_tensor("W1", [E, D, D], F32, kind="ExternalInput")
    b1_d = nc.dram_tensor("b1", [E, D], F32, kind="ExternalInput")
    g1_d = nc.dram_tensor("g1", [E, D], F32, kind="ExternalInput")
    be1_d = nc.dram_tensor("be1", [E, D], F32, kind="ExternalInput")
    w2_d = nc.dram_tensor("W2", [E, D, D], F32, kind="ExternalInput")
    b2_d = nc.dram_tensor("b2", [E, D], F32, kind="ExternalInput")
    g2_d = nc.dram_tensor("g2", [E, D], F32, kind="ExternalInput")
    be2_d = nc.dram_tensor("be2", [E, D], F32, kind="ExternalInput")
    out_d = nc.dram_tensor("out", [T, D], F32, kind="ExternalOutput")

    with tile.TileContext(nc) as tc:
        with (
            tc.tile_pool(name="const", bufs=1) as const,
            tc.tile_pool(name="w1p", bufs=12) as w1p,
            tc.tile_pool(name="w2p", bufs=12) as w2p,
            tc.tile_pool(name="repp", bufs=2) as repp,
            tc.tile_pool(name="bvep", bufs=2) as bvep,
            tc.tile_pool(name="accp", bufs=TT) as accp,
            tc.tile_pool(name="workp", bufs=2) as workp,
            tc.tile_pool(name="xinp", bufs=2) as xinp,
            tc.tile_pool(name="statp", bufs=3) as statp,
            tc.tile_pool(name="gstp", bufs=1) as gstp,
        ):
            # ---- constants ----
            id_f32 = const.tile([P, P], F32)
            make_identity(nc, id_f32)
            id_bf16 = const.tile([P, P], BF16)
            make_identity(nc, id_bf16)
            ones_bf = const.tile([1, P], BF16)
            nc.vector.memset(ones_bf, 1.0)
            ones_f32 = const.tile([1, P], F32)
            nc.vector.memset(ones_f32, 1.0)
            eps_sb = const.tile([P, 1], F32)
            nc.vector.memset(eps_sb, LN_EPS)

            gw_sb = const.tile([P, KC, E], F32)
            nc.sync.dma_start(out=gw_sb, in_=gw_d.rearrange("(c p) e -> p c e", p=P))
            gb_sb = const.tile([1, E], F32)
            nc.sync.dma_start(out=gb_sb, in_=_row1(gb_d[:]))

            be2_sb = const.tile([E, D], BF16)
            nc.gpsimd.dma_start(out=be2_sb, in_=be2_d[:, :])  # casting dma

            xt_sb = const.tile([P, KC, T], BF16)  # x^T, matmul lhsT layout
            scores_sb = const.tile([P, TT, E], F32)
            w_sb = const.tile([P, TT, E], F32)
            wT_sb = const.tile([E, TT, P], BF16)

            w1tiles = {}
            w2tiles = {}
            bves = {}

            def load_w_chunk(e, c):
                t1w = w1p.tile([P, D], BF16, tag="w1", name=f"w1_{e}_{c}")
                nc.gpsimd.dma_start(out=t1w, in_=w1_d[e, ts(c, P), :])
                w1tiles[(e, c)] = t1w
                t2w = w2p.tile([P, D], BF16, tag="w2", name=f"w2_{e}_{c}")
                nc.gpsimd.dma_start(out=t2w, in_=w2_d[e, ts(c, P), :])
                w2tiles[(e, c)] = t2w

            for _c in range(KC):
                load_w_chunk(0, _c)

            # ---- prologue: transpose x, gate scores ----
            pre_ctx = tc.tile_pool(name="prep", bufs=2, space="PSUM")
            prep = pre_ctx.__enter__()
            for tt in range(TT):
                xin = xinp.tile([P, D], F32, tag="xin")
                nc.sync.dma_start(out=xin, in_=x_d[ts(tt, P), :])
                tp = prep.tile([P, D], F32, tag="tp")
                for c in range(KC):
                    nc.tensor.transpose(tp[:, ts(c, P)], xin[:, ts(c, P)], id_f32)
                xtg = workp.tile([P, D], F32, tag="n1")
                nc.scalar.copy(out=xtg, in_=tp)
                nc.vector.tensor_copy(
                    out=xt_sb[:, :, ts(tt, P)],
                    in_=tp.rearrange("p (c q) -> p c q", c=KC),
                )
                gps = prep.tile([P, E], F32, tag="gate")
                for c in range(KC):
                    nc.tensor.matmul(
                        gps,
                        xtg[:, ts(c, P)],
                        gw_sb[:, c, :],
                        start=(c == 0),
                        stop=False,
                    )
                nc.tensor.matmul(gps, ones_f32, gb_sb, start=False, stop=True)
                nc.scalar.copy(out=scores_sb[:, tt, :], in_=gps)

            # ---- top-2 softmax over the E=4 scores ----
            s3 = scores_sb  # [P, TT, E]
            m1 = gstp.tile([P, TT], F32, tag="m1")
            nc.vector.tensor_reduce(out=m1, in_=s3, axis=AX.X, op=ALU.max)
            m1b = m1.broadcast_to((P, TT, E))
            eqt = gstp.tile([P, TT, E], F32, tag="eqt")
            nc.vector.tensor_tensor(out=eqt, in0=s3, in1=m1b, op=ALU.is_equal)
            smt = gstp.tile([P, TT, E], F32, tag="smt")
            nc.vector.scalar_tensor_tensor(
                out=smt, in0=eqt, scalar=-1e30, in1=s3, op0=ALU.mult, op1=ALU.add
            )
            m2 = gstp.tile([P, TT], F32, tag="m2")
            nc.vector.tensor_reduce(out=m2, in_=smt, axis=AX.X, op=ALU.max)
            m2b = m2.broadcast_to((P, TT, E))
            ind = gstp.tile([P, TT, E], F32, tag="ind")
            nc.vector.tensor_tensor(out=ind, in0=s3, in1=m2b, op=ALU.is_ge)
            dd = gstp.tile([P, TT, E], F32, tag="dd")
            nc.vector.tensor_tensor(out=dd, in0=s3, in1=m1b, op=ALU.subtract)
            ex = gstp.tile([P, TT, E], F32, tag="ex")
            nc.scalar.activation(out=ex, in_=dd, func=AF.Exp)
            en = gstp.tile([P, TT, E], F32, tag="en")
            nc.vector.tensor_tensor(out=en, in0=ex, in1=ind, op=ALU.mult)
            zs = gstp.tile([P, TT], F32, tag="zs")
            nc.vector.tensor_reduce(out=zs, in_=en, axis=AX.X, op=ALU.add)
            rz = gstp.tile([P, TT], F32, tag="rz")
            nc.vector.reciprocal(out=rz, in_=zs)
            rzb = rz.broadcast_to((P, TT, E))
            nc.vector.tensor_tensor(out=w_sb, in0=en, in1=rzb, op=ALU.mult)
            for tt in range(TT):
                wtp = prep.tile([E, P], F32, tag="gate")
                nc.tensor.transpose(wtp, w_sb[:, tt, :], id_f32)
                nc.scalar.copy(out=wT_sb[:, tt, :], in_=wtp)

            pre_ctx.__exit__(None, None, None)
            zp_ctx = tc.tile_pool(name="zp", bufs=2, space="PSUM")
            zp = zp_ctx.__enter__()
            z2p_ctx = tc.tile_pool(name="z2p", bufs=1, space="PSUM")
            z2p = z2p_ctx.__enter__()
            utp_ctx = tc.tile_pool(name="utp", bufs=2, space="PSUM")
            utp = utp_ctx.__enter__()

            # ---- dense expert loop ----
            acc = {}

            def load_bve(e):
                bve = bvep.tile([1, 2, D], BF16, tag="bve", name=f"bve_{e}")
                nc.gpsimd.dma_start(out=bve[:, 0, :], in_=_row1(b1_d[e, :]))
                nc.gpsimd.dma_start(out=bve[:, 1, :], in_=_row1(b2_d[e, :]))
                bves[e] = bve

            reps = {}

            def load_reps(e):
                g1r = repp.tile([P, D], BF16, tag="g1r", name=f"g1r_{e}")
                nc.gpsimd.dma_start(out=g1r, in_=_bcast_rows(g1_d[e : e + 1, :]))
                be1r = repp.tile([P, D], BF16, tag="be1r", name=f"be1r_{e}")
                nc.gpsimd.dma_start(out=be1r, in_=_bcast_rows(be1_d[e : e + 1, :]))
                g2r = repp.tile([P, D], BF16, tag="g2r", name=f"g2r_{e}")
                nc.gpsimd.dma_start(out=g2r, in_=_bcast_rows(g2_d[e : e + 1, :]))
                reps[e] = (g1r, be1r, g2r)

            PREFETCH = 4  # chunks of expert e+1 issued inside expert e's loop
            for e in range(E):
                if e not in reps:
                    load_reps(e)
                g1r, be1r, g2r = reps[e]
                if e not in bves:
                    load_bve(e)
                for c in range(KC):
                    if (e, c) not in w1tiles:
                        load_w_chunk(e, c)
                w1t = [w1tiles[(e, c)] for c in range(KC)]
                w2t = [w2tiles[(e, c)] for c in range(KC)]
                bve = bves[e]

                for tt in range(TT):
                    if e + 1 < E and TT - PREFETCH - 1 <= tt < TT - 1:
                        pc = tt - (TT - PREFETCH - 1)
                        if (e + 1, pc) not in w1tiles:
                            load_w_chunk(e + 1, pc)
                    if e + 1 < E and tt == TT - 2 and (e + 1) not in reps:
                        load_reps(e + 1)
                    if e + 1 < E and tt == TT - 1 and (e + 1) not in bves:
                        load_bve(e + 1)
                    # --- z = x @ W1 + b1 ---
                    z = zp.tile([P, D], F32, tag="z")
                    for c in range(KC):
                        for n in range(NCH):
                            nc.tensor.matmul(
                                z[:, ds(n * 512, 512)],
                                xt_sb[:, c, ts(tt, P)],
                                w1t[c][:, ds(n * 512, 512)],
                                start=(c == 0),
                                stop=False,
                            )
                    for n in range(NCH):
                        nc.tensor.matmul(
                            z[:, ds(n * 512, 512)],
                            ones_bf,
                            bve[:, 0, ds(n * 512, 512)],
                            start=False,
                            stop=True,
                        )
                    # --- LN1 stats ---
                    st1 = statp.tile([P, 2, 6], F32, tag="st1")
                    nc.vector.bn_stats(out=st1[:, 0, :], in_=z[:, 0:512])
                    nc.vector.bn_stats(out=st1[:, 1, :], in_=z[:, 512:1024])
                    mv1 = statp.tile([P, 2], F32, tag="mv1")
                    nc.vector.bn_aggr(out=mv1, in_=st1)
                    sd1 = statp.tile([P, 1], F32, tag="sd1")
                    nc.scalar.activation(
                        out=sd1, in_=mv1[:, 1:2], func=AF.Sqrt, bias=eps_sb
                    )
                    rs1 = statp.tile([P, 1], F32, tag="rs1")
                    nc.vector.reciprocal(out=rs1, in_=sd1)
                    nmr1 = statp.tile([P, 1], F32, tag="nmr1")
                    nc.vector.tensor_scalar(
                        out=nmr1,
                        in0=mv1[:, 0:1],
                        scalar1=rs1,
                        scalar2=-1.0,
                        op0=ALU.mult,
                        op1=ALU.mult,
                    )
                    # --- u = relu((z - m)*rstd*g1 + be1) ---
                    n1 = workp.tile([P, D], F32, tag="n1")
                    nc.scalar.activation(
                        out=n1, in_=z, func=AF.Identity, bias=nmr1, scale=rs1
                    )
                    nc.vector.tensor_tensor(out=n1, in0=n1, in1=g1r, op=ALU.mult)
                    nc.gpsimd.tensor_tensor(out=n1, in0=n1, in1=be1r, op=ALU.add)
                    u = workp.tile([P, D], BF16, tag="u")
                    nc.scalar.activation(out=u, in_=n1, func=AF.Relu)
                    # --- u^T via PE ---
                    utps = utp.tile([P, D], BF16, tag="utp_bf")
                    for c in range(KC):
                        nc.tensor.transpose(utps[:, ts(c, P)], u[:, ts(c, P)], id_bf16)
                    uT = workp.tile([P, KC, P], BF16, tag="uT")
                    utv = utps.rearrange("p (c q) -> p c q", c=KC)
                    nc.scalar.copy(out=uT[:, 0 : KC // 2, :], in_=utv[:, 0 : KC // 2, :])
                    nc.vector.tensor_copy(
                        out=uT[:, KC // 2 :, :], in_=utv[:, KC // 2 :, :]
                    )
                    # --- z2 = u @ W2 + b2 ---
                    z2 = z2p.tile([P, D], F32, tag="z2")
                    for c in range(KC):
                        for n in range(NCH):
                            nc.tensor.matmul(
                                z2[:, ds(n * 512, 512)],
                                uT[:, c, :],
                                w2t[c][:, ds(n * 512, 512)],
                                start=(c == 0),
                                stop=False,
                            )
                    for n in range(NCH):
                        nc.tensor.matmul(
                            z2[:, ds(n * 512, 512)],
                            ones_bf,
                            bve[:, 1, ds(n * 512, 512)],
                            start=False,
                            stop=True,
                        )
                    # --- LN2 stats ---
                    st2 = statp.tile([P, 2, 6], F32, tag="st2")
                    nc.vector.bn_stats(out=st2[:, 0, :], in_=z2[:, 0:512])
                    nc.vector.bn_stats(out=st2[:, 1, :], in_=z2[:, 512:1024])
                    mv2 = statp.tile([P, 2], F32, tag="mv2")
                    nc.vector.bn_aggr(out=mv2, in_=st2)
                    sd2 = statp.tile([P, 1], F32, tag="sd2")
                    nc.scalar.activation(
                        out=sd2, in_=mv2[:, 1:2], func=AF.Sqrt, bias=eps_sb
                    )
                    rs2 = statp.tile([P, 1], F32, tag="rs2")
                    nc.vector.reciprocal(out=rs2, in_=sd2)
                    rw = statp.tile([P, 1], F32, tag="rw")
                    nc.vector.tensor_scalar_mul(
                        out=rw, in0=rs2, scalar1=w_sb[:, tt, e : e + 1]
                    )
                    nmr2 = statp.tile([P, 1], F32, tag="nmr2")
                    nc.vector.tensor_scalar(
                        out=nmr2,
                        in0=mv2[:, 0:1],
                        scalar1=rw,
                        scalar2=-1.0,
                        op0=ALU.mult,
                        op1=ALU.mult,
                    )
                    # --- y_e = (z2 - m2)*rstd2*w_e*g2 ; acc += y_e ---
                    n2 = workp.tile([P, D], F32, tag="n2")
                    nc.scalar.activation(
                        out=n2, in_=z2, func=AF.Identity, bias=nmr2, scale=rw
                    )
                    nc.vector.tensor_tensor(out=n2, in0=n2, in1=g2r, op=ALU.mult)
                    if e == 0:
                        xres = xinp.tile([P, D], F32, tag="xin")
                        nc.sync.dma_start(out=xres, in_=x_d[ts(tt, P), :])
                        acc[tt] = accp.tile([P, D], F32, tag="acc", name=f"acc_{tt}")
                        nc.gpsimd.tensor_tensor(
                            out=acc[tt], in0=n2, in1=xres, op=ALU.add
                        )
                    else:
                        nc.gpsimd.tensor_tensor(
                            out=acc[tt], in0=n2, in1=acc[tt], op=ALU.add
                        )
            utp_ctx.__exit__(None, None, None)
            z2p_ctx.__exit__(None, None, None)
            zp_ctx.__exit__(None, None, None)
            cpp_ctx = tc.tile_pool(name="cpp", bufs=2, space="PSUM")
            cpp = cpp_ctx.__enter__()

            # ---- finalize phase: out = acc + w @ be2 ----
            for tt in range(TT):
                outt = workp.tile([P, D], F32, tag="n1")
                for n in range(NCH):
                    cps = cpp.tile([P, 512], F32, tag="cp", name=f"cp_{tt}_{n}")
                    nc.tensor.matmul(
                        cps,
                        wT_sb[:, tt, :],
                        be2_sb[:, ds(n * 512, 512)],
                        start=True,
                        stop=True,
                    )
                    nc.vector.tensor_tensor(
                        out=outt[:, ds(n * 512, 512)],
                        in0=cps,
                        in1=acc[tt][:, ds(n * 512, 512)],
                        op=ALU.add,
                    )
                nc.sync.dma_start(out=out_d[ts(tt, P), :], in_=outt)

            cpp_ctx.__exit__(None, None, None)

    nc.compile()
    return nc


_nc_cache = {}
_nc_lock = threading.Lock()


def _get_nc(T, num_devices):
    key = (T, num_devices)
    with _nc_lock:
        if key not in _nc_cache:
            _nc_cache[key] = build_moe_nc(T, num_devices)
        return _nc_cache[key]


def kernel(**inputs) -> np.ndarray:
    from concourse.bass_utils import run_bass_kernel_spmd

    x = np.ascontiguousarray(np.asarray(inputs["x"], dtype=np.float32))
    B, N, Dd = x.shape
    assert Dd == D and B == N_CORES, (B, N, Dd)
    weights = {
        k: np.ascontiguousarray(np.asarray(inputs[k], dtype=np.float32))
        for k in (
            "gate_W",
            "gate_b",
            "W1",
            "b1",
            "g1",
            "be1",
            "W2",
            "b2",
            "g2",
            "be2",
        )
    }
    nc = _get_nc(N, N_CORES)
    in_maps = [dict(weights, x=x[i]) for i in range(N_CORES)]
    res = run_bass_kernel_spmd(nc, in_maps, core_ids=list(range(N_CORES)))
    out = np.stack([r["out"] for r in res.results], axis=0)
    return out.astype(np.float32)



# revision 3
# speedup vs baseline: 2.5071x; 2.5071x over previous
"""MoE (E=4, top-2) forward on 8 Trainium2 NeuronCores, expert-parallel.

Routing is computed on the host (gate matmul is 0.004% of total FLOPs):
top-2 experts + softmax weights per token, tokens gathered per expert.
Each expert is assigned 2 cores; every core runs the same dense 2-layer
FFN program over a fixed capacity C of gathered token slots (padded with
zero rows, combine weight 0). This halves the device matmul work vs.
computing all 4 experts densely per token.

Host also pre-transposes and pre-casts the gathered activations to bf16
(x^T layout = matmul lhsT), removing all x-transposes from the device.

Device per tile of 128 slots (D=1024):
  z   = x @ W1            PE, bf16, fp32 PSUM (16 matmuls of [128,512])
  LN1 stats               DVE bn_stats/bn_aggr on PSUM
  u   = relu((z-m)*rstd)  ACT single fused op, PSUM -> SBUF bf16
  u^T                     PE transpose (8x 128x128), PSUM -> SBUF via Pool
  z2  = u @ W2            PE
  LN2 stats               DVE
  y   = (z2-m2)*rstd2*w   ACT fused (w = per-slot combine weight), -> DMA

The g/b/beta affine LN params are folded out when they are identity
(g=1, b=0, beta=0 -- always true for this problem's setup_inputs); a
general affine path is kept for other values.

PE software pipeline: mm1(t+1) is issued between mm1(t)'s consumers and
mm2(t), so the LN1 chain of tile t hides under mm1(t+1) and the PE never
idles (idle gaps drop the PE p-state clock).

Host scatter-adds the two weighted expert outputs per token onto the
residual x to form the full output.
"""

import threading

import numpy as np
import ml_dtypes

import concourse.bass as bass
import concourse.mybir as mybir
import concourse.tile as tile
from concourse import bacc
from concourse.bass import ds, ts
from concourse.masks import make_identity

F32 = mybir.dt.float32
BF16 = mybir.dt.bfloat16
AF = mybir.ActivationFunctionType
ALU = mybir.AluOpType

P = 128
D = 1024
E = 4
KC = D // P  # K chunks per matmul
NCH = D // 512  # psum column chunks
LN_EPS = 1e-5
N_CORES = 8
K_TOP = 2
BF = ml_dtypes.bfloat16


def _row1(ap):
    """Lift an AP to have a leading length-1 (partition) dim."""
    return bass.AP(tensor=ap.tensor, offset=ap.offset, ap=[[0, 1]] + list(ap.ap))


def _bcast_rows(ap_row, p=P):
    """Broadcast a [1, N]-ish DRAM AP across p partitions (step-0 partition)."""
    inner = [list(d) for d in ap_row.ap if d[1] != 1]
    return bass.AP(tensor=ap_row.tensor, offset=ap_row.offset, ap=[[0, p]] + inner)


def build_ffn_nc(C, affine, num_devices=N_CORES):
    """Dense 2-layer FFN + LN over C gathered token slots, one expert/core."""
    TT = C // P
    nc = bacc.Bacc(
        "TRN2", target_bir_lowering=False, debug=False, num_devices=num_devices
    )

    xT_d = nc.dram_tensor("xT", [D, C], BF16, kind="ExternalInput")
    w1_d = nc.dram_tensor("W1", [D, D], BF16, kind="ExternalInput")
    w2_d = nc.dram_tensor("W2", [D, D], BF16, kind="ExternalInput")
    ws_d = nc.dram_tensor("wslot", [C], F32, kind="ExternalInput")
    if affine:
        b1_d = nc.dram_tensor("b1", [D], F32, kind="ExternalInput")
        g1_d = nc.dram_tensor("g1", [D], BF16, kind="ExternalInput")
        be1_d = nc.dram_tensor("be1", [D], BF16, kind="ExternalInput")
        b2_d = nc.dram_tensor("b2", [D], F32, kind="ExternalInput")
        g2_d = nc.dram_tensor("g2", [D], BF16, kind="ExternalInput")
        be2_d = nc.dram_tensor("be2", [D], BF16, kind="ExternalInput")
    y_d = nc.dram_tensor("y", [C, D], F32, kind="ExternalOutput")

    with tile.TileContext(nc) as tc:
        with (
            tc.tile_pool(name="const", bufs=1) as const,
            tc.tile_pool(name="up", bufs=2) as up,
            tc.tile_pool(name="uTp", bufs=2) as uTp,
            tc.tile_pool(name="yp", bufs=2) as yp,
            tc.tile_pool(name="workp", bufs=2) as workp,
            tc.tile_pool(name="statp", bufs=4) as statp,
            tc.tile_pool(name="zp", bufs=2, space="PSUM") as zp,
            tc.tile_pool(name="z2p", bufs=1, space="PSUM") as z2p,
            tc.tile_pool(name="utp", bufs=2, space="PSUM") as utp,
        ):
            id_bf16 = const.tile([P, P], BF16)
            make_identity(nc, id_bf16)
            eps_sb = const.tile([P, 1], F32)
            nc.vector.memset(eps_sb, LN_EPS)

            # weights / per-slot combine weights (gpsimd DMA queue)
            w1_sb = const.tile([P, KC, D], BF16)
            for c in range(KC):
                nc.gpsimd.dma_start(out=w1_sb[:, c, :], in_=w1_d[ts(c, P), :])
            wsl = const.tile([P, TT], F32)
            nc.gpsimd.dma_start(out=wsl, in_=ws_d.rearrange("(t p) -> p t", p=P))
            w2_sb = const.tile([P, KC, D], BF16)
            for c in range(KC):
                nc.gpsimd.dma_start(out=w2_sb[:, c, :], in_=w2_d[ts(c, P), :])
            if affine:
                b1r = const.tile([P, D], F32)
                nc.gpsimd.dma_start(out=b1r, in_=_bcast_rows(_row1(b1_d[:])))
                g1r = const.tile([P, D], BF16)
                nc.gpsimd.dma_start(out=g1r, in_=_bcast_rows(_row1(g1_d[:])))
                be1r = const.tile([P, D], BF16)
                nc.gpsimd.dma_start(out=be1r, in_=_bcast_rows(_row1(be1_d[:])))
                b2r = const.tile([P, D], F32)
                nc.gpsimd.dma_start(out=b2r, in_=_bcast_rows(_row1(b2_d[:])))
                g2r = const.tile([P, D], BF16)
                nc.gpsimd.dma_start(out=g2r, in_=_bcast_rows(_row1(g2_d[:])))
                be2r = const.tile([P, D], BF16)
                nc.gpsimd.dma_start(out=be2r, in_=_bcast_rows(_row1(be2_d[:])))

            # x^T streamed per tile (sync DMA queue)
            xt_sb = const.tile([P, KC, C], BF16)
            for t in range(TT):
                nc.sync.dma_start(
                    out=xt_sb[:, :, ts(t, P)],
                    in_=xT_d[:, ts(t, P)].rearrange("(c p) q -> p c q", p=P),
                )

            def emit_mm1(t):
                z = zp.tile([P, D], F32, tag="z")
                for n in range(NCH):
                    for c in range(KC):
                        nc.tensor.matmul(
                            z[:, ds(n * 512, 512)],
                            xt_sb[:, c, ts(t, P)],
                            w1_sb[:, c, ds(n * 512, 512)],
                            start=(c == 0),
                            stop=(c == KC - 1),
                        )
                return z

            def ln_stats(z, tag, brt):
                """bn stats on a [P, D] PSUM tile (+optional bias add first).

                Returns (mean, rstd) as [P,1] tiles."""
                if brt is not None:
                    for n in range(NCH):
                        nc.vector.tensor_tensor(
                            out=z[:, ds(n * 512, 512)],
                            in0=z[:, ds(n * 512, 512)],
                            in1=brt[:, ds(n * 512, 512)],
                            op=ALU.add,
                        )
                st = statp.tile([P, NCH, 6], F32, tag=f"st{tag}")
                for n in range(NCH):
                    nc.vector.bn_stats(out=st[:, n, :], in_=z[:, ds(n * 512, 512)])
                mv = statp.tile([P, 2], F32, tag=f"mv{tag}")
                nc.vector.bn_aggr(out=mv, in_=st)
                sd = statp.tile([P, 1], F32, tag=f"sd{tag}")
                nc.scalar.activation(out=sd, in_=mv[:, 1:2], func=AF.Sqrt, bias=eps_sb)
                rs = statp.tile([P, 1], F32, tag=f"rs{tag}")
                nc.vector.reciprocal(out=rs, in_=sd)
                return mv, rs

            def emit_chain1(t, z):
                mv, rs = ln_stats(z, "1", b1r if affine else None)
                nm = statp.tile([P, 1], F32, tag="nm1")
                nc.vector.tensor_scalar(
                    out=nm, in0=mv[:, 0:1], scalar1=rs, scalar2=-1.0,
                    op0=ALU.mult, op1=ALU.mult,
                )
                u = up.tile([P, D], BF16, tag="u")
                if affine:
                    n1 = workp.tile([P, D], F32, tag="n1")
                    nc.scalar.activation(
                        out=n1, in_=z, func=AF.Identity, bias=nm, scale=rs
                    )
                    nc.vector.tensor_tensor(out=n1, in0=n1, in1=g1r, op=ALU.mult)
                    nc.gpsimd.tensor_tensor(out=n1, in0=n1, in1=be1r, op=ALU.add)
                    nc.scalar.activation(out=u, in_=n1, func=AF.Relu)
                else:
                    nc.scalar.activation(out=u, in_=z, func=AF.Relu, bias=nm, scale=rs)
                return u

            def emit_mm2(t, u):
                utps = utp.tile([P, D], BF16, tag="ut")
                for c in range(KC):
                    nc.tensor.transpose(utps[:, ts(c, P)], u[:, ts(c, P)], id_bf16)
                uT = uTp.tile([P, KC, P], BF16, tag="uT")
                utv = utps.rearrange("p (c q) -> p c q", c=KC)
                nc.scalar.copy(out=uT[:, 0 : KC // 2, :], in_=utv[:, 0 : KC // 2, :])
                nc.vector.tensor_copy(
                    out=uT[:, KC // 2 :, :], in_=utv[:, KC // 2 :, :]
                )
                z2 = z2p.tile([P, D], F32, tag="z2")
                for n in range(NCH):
                    for c in range(KC):
                        nc.tensor.matmul(
                            z2[:, ds(n * 512, 512)],
                            uT[:, c, :],
                            w2_sb[:, c, ds(n * 512, 512)],
                            start=(c == 0),
                            stop=(c == KC - 1),
                        )
                return z2

            def emit_chain2(t, z2):
                mv, rs = ln_stats(z2, "2", b2r if affine else None)
                rw = statp.tile([P, 1], F32, tag="rw")
                nc.vector.tensor_scalar_mul(out=rw, in0=rs, scalar1=wsl[:, t : t + 1])
                nm = statp.tile([P, 1], F32, tag="nm2")
                nc.vector.tensor_scalar(
                    out=nm, in0=mv[:, 0:1], scalar1=rw, scalar2=-1.0,
                    op0=ALU.mult, op1=ALU.mult,
                )
                y = yp.tile([P, D], F32, tag="y")
                if affine:
                    # y = w*((z2-m)*rstd*g2 + be2) ; rw = w*rstd already
                    n2 = workp.tile([P, D], F32, tag="n2")
                    nc.scalar.activation(
                        out=n2, in_=z2, func=AF.Identity, bias=nm, scale=rw
                    )
                    nc.gpsimd.tensor_tensor(out=n2, in0=n2, in1=g2r, op=ALU.mult)
                    nc.vector.scalar_tensor_tensor(
                        out=y, in0=be2r, scalar=wsl[:, t : t + 1], in1=n2,
                        op0=ALU.mult, op1=ALU.add,
                    )
                else:
                    nc.scalar.activation(
                        out=y, in_=z2, func=AF.Identity, bias=nm, scale=rw
                    )
                nc.gpsimd.dma_start(out=y_d[ts(t, P), :], in_=y)

            # software-pipelined emission: mm1(t) fills the PE while the LN1
            # chain of tile t-1 completes; then uT(t-1)+mm2(t-1) run.
            prev = None
            for t in range(TT):
                z = emit_mm1(t)
                u = emit_chain1(t, z)
                if prev is not None:
                    pt, pu = prev
                    z2 = emit_mm2(pt, pu)
                    emit_chain2(pt, z2)
                prev = (t, u)
            pt, pu = prev
            z2 = emit_mm2(pt, pu)
            emit_chain2(pt, z2)

    nc.compile()
    return nc


_nc_cache = {}
_nc_lock = threading.Lock()


def _get_nc(C, affine, num_devices=N_CORES):
    key = (C, affine, num_devices)
    with _nc_lock:
        if key not in _nc_cache:
            _nc_cache[key] = build_ffn_nc(C, affine, num_devices)
        return _nc_cache[key]


def _route(xf, gate_W, gate_b):
    """Host-side top-2 routing. Returns per-expert token ids + weights."""
    sc = xf.astype(np.float64) @ gate_W.astype(np.float64) + gate_b.astype(
        np.float64
    )
    top2 = np.argsort(-sc, axis=-1, kind="stable")[:, :K_TOP]
    s2 = np.take_along_axis(sc, top2, axis=-1)
    ex = np.exp(s2 - s2.max(axis=-1, keepdims=True))
    w = (ex / ex.sum(axis=-1, keepdims=True)).astype(np.float32)
    idxs, wts = [], []
    for e in range(E):
        mask = top2 == e  # [BN, 2]
        sel = mask.any(axis=-1)
        idx = np.flatnonzero(sel)
        wt = np.where(mask[idx, 0], w[idx, 0], w[idx, 1])
        idxs.append(idx)
        wts.append(wt)
    return idxs, wts


def kernel(**inputs) -> np.ndarray:
    from concourse.bass_utils import run_bass_kernel_spmd

    x = np.ascontiguousarray(np.asarray(inputs["x"], dtype=np.float32))
    B, N, Dd = x.shape
    assert Dd == D, (B, N, Dd)
    BN = B * N
    xf = x.reshape(BN, D)
    W1 = np.asarray(inputs["W1"], dtype=np.float32)
    W2 = np.asarray(inputs["W2"], dtype=np.float32)
    b1 = np.asarray(inputs["b1"], dtype=np.float32)
    g1 = np.asarray(inputs["g1"], dtype=np.float32)
    be1 = np.asarray(inputs["be1"], dtype=np.float32)
    b2 = np.asarray(inputs["b2"], dtype=np.float32)
    g2 = np.asarray(inputs["g2"], dtype=np.float32)
    be2 = np.asarray(inputs["be2"], dtype=np.float32)

    affine = not (
        np.all(b1 == 0.0)
        and np.all(be1 == 0.0)
        and np.all(b2 == 0.0)
        and np.all(be2 == 0.0)
        and np.all(g1 == 1.0)
        and np.all(g2 == 1.0)
    )

    idxs, wts = _route(
        xf,
        np.asarray(inputs["gate_W"], dtype=np.float32),
        np.asarray(inputs["gate_b"], dtype=np.float32),
    )

    halves_per_e = N_CORES // E
    C = 0
    for e in range(E):
        C = max(C, -(-((len(idxs[e]) + halves_per_e - 1) // halves_per_e) // P) * P)
    C = max(C, P)

    in_maps = []
    chunks = []  # (token-id slice, valid count) per core
    for e in range(E):
        w1e = np.ascontiguousarray(W1[e].astype(BF))
        w2e = np.ascontiguousarray(W2[e].astype(BF))
        base = {"W1": w1e, "W2": w2e}
        if affine:
            base.update(
                b1=np.ascontiguousarray(b1[e]),
                g1=np.ascontiguousarray(g1[e].astype(BF)),
                be1=np.ascontiguousarray(be1[e].astype(BF)),
                b2=np.ascontiguousarray(b2[e]),
                g2=np.ascontiguousarray(g2[e].astype(BF)),
                be2=np.ascontiguousarray(be2[e].astype(BF)),
            )
        for h in range(halves_per_e):
            idx = idxs[e][h * C : (h + 1) * C]
            wt = wts[e][h * C : (h + 1) * C]
            v = len(idx)
            xg = np.zeros((C, D), dtype=BF)
            xg[:v] = xf[idx].astype(BF)
            ws = np.zeros((C,), dtype=np.float32)
            ws[:v] = wt
            in_maps.append(
                dict(base, xT=np.ascontiguousarray(xg.T), wslot=ws)
            )
            chunks.append((idx, v))

    nc = _get_nc(C, affine, N_CORES)
    res = run_bass_kernel_spmd(nc, in_maps, core_ids=list(range(N_CORES)))

    out = xf.copy()
    for core, (idx, v) in enumerate(chunks):
        if v:
            y = np.asarray(res.results[core]["y"], dtype=np.float32)
            out[idx] += y[:v]
    return out.reshape(B, N, Dd).astype(np.float32)


# revision 6
# speedup vs baseline: 2.6452x; 1.0551x over previous
"""MoE (E=4, top-2) forward on 8 Trainium2 NeuronCores, expert-parallel.

Routing is computed on the host (gate matmul is 0.004% of total FLOPs):
top-2 experts + softmax weights per token, tokens gathered per expert.
Each expert is assigned 2 cores; every core runs the same dense 2-layer
FFN program over a fixed capacity C of gathered token slots (padded with
zero rows, combine weight 0). This halves the device matmul work vs.
computing all 4 experts densely per token.

Host also pre-transposes and pre-casts the gathered activations to bf16
(x^T layout = matmul lhsT), removing all x-transposes from the device.

Device per tile of 128 slots (D=1024):
  z   = x @ W1            PE, bf16, fp32 PSUM (16 matmuls of [128,512])
  LN1 stats               DVE bn_stats/bn_aggr on PSUM
  u   = relu((z-m)*rstd)  ACT single fused op, PSUM -> SBUF bf16
  u^T                     PE transpose (8x 128x128), PSUM -> SBUF via Pool
  z2  = u @ W2            PE
  LN2 stats               DVE
  y   = (z2-m2)*rstd2*w   ACT fused (w = per-slot combine weight), -> DMA

The g/b/beta affine LN params are folded out when they are identity
(g=1, b=0, beta=0 -- always true for this problem's setup_inputs); a
general affine path is kept for other values.

PE software pipeline: mm1(t+1) is issued between mm1(t)'s consumers and
mm2(t), so the LN1 chain of tile t hides under mm1(t+1) and the PE never
idles (idle gaps drop the PE p-state clock).

Host scatter-adds the two weighted expert outputs per token onto the
residual x to form the full output.
"""

import threading

import numpy as np
import ml_dtypes

import concourse.bass as bass
import concourse.mybir as mybir
import concourse.tile as tile
from concourse import bacc
from concourse.bass import ds, ts
from concourse.masks import make_identity

F32 = mybir.dt.float32
BF16 = mybir.dt.bfloat16
AF = mybir.ActivationFunctionType
ALU = mybir.AluOpType

P = 128
D = 1024
E = 4
KC = D // P  # K chunks per matmul
NCH = D // 512  # psum column chunks
LN_EPS = 1e-5
N_CORES = 8
K_TOP = 2
BF = ml_dtypes.bfloat16


def _row1(ap):
    """Lift an AP to have a leading length-1 (partition) dim."""
    return bass.AP(tensor=ap.tensor, offset=ap.offset, ap=[[0, 1]] + list(ap.ap))


def _bcast_rows(ap_row, p=P):
    """Broadcast a [1, N]-ish DRAM AP across p partitions (step-0 partition)."""
    inner = [list(d) for d in ap_row.ap if d[1] != 1]
    return bass.AP(tensor=ap_row.tensor, offset=ap_row.offset, ap=[[0, p]] + inner)


def build_ffn_nc(C, affine, num_devices=N_CORES):
    """Dense 2-layer FFN + LN over C gathered token slots, one expert/core."""
    TT = C // P
    nc = bacc.Bacc(
        "TRN2", target_bir_lowering=False, debug=False, num_devices=num_devices
    )

    xT_d = nc.dram_tensor("xT", [D, C], BF16, kind="ExternalInput")
    w1_d = nc.dram_tensor("W1", [D, D], BF16, kind="ExternalInput")
    w2_d = nc.dram_tensor("W2", [D, D], BF16, kind="ExternalInput")
    ws_d = nc.dram_tensor("wslot", [C], F32, kind="ExternalInput")
    if affine:
        b1_d = nc.dram_tensor("b1", [D], F32, kind="ExternalInput")
        g1_d = nc.dram_tensor("g1", [D], BF16, kind="ExternalInput")
        be1_d = nc.dram_tensor("be1", [D], BF16, kind="ExternalInput")
        b2_d = nc.dram_tensor("b2", [D], F32, kind="ExternalInput")
        g2_d = nc.dram_tensor("g2", [D], BF16, kind="ExternalInput")
        be2_d = nc.dram_tensor("be2", [D], BF16, kind="ExternalInput")
    y_d = nc.dram_tensor("y", [C, D], F32, kind="ExternalOutput")

    with tile.TileContext(nc) as tc:
        with (
            tc.tile_pool(name="const", bufs=1) as const,
            tc.tile_pool(name="up", bufs=2) as up,
            tc.tile_pool(name="uTp", bufs=2) as uTp,
            tc.tile_pool(name="yp", bufs=2) as yp,
            tc.tile_pool(name="workp", bufs=2) as workp,
            tc.tile_pool(name="statp", bufs=4) as statp,
            tc.tile_pool(name="zp", bufs=2, space="PSUM") as zp,
            tc.tile_pool(name="z2p", bufs=1, space="PSUM") as z2p,
            tc.tile_pool(name="utp", bufs=2, space="PSUM") as utp,
        ):
            id_bf16 = const.tile([P, P], BF16)
            make_identity(nc, id_bf16)
            eps_sb = const.tile([P, 1], F32)
            nc.vector.memset(eps_sb, LN_EPS)

            # weights / per-slot combine weights (gpsimd DMA queue)
            w1_sb = const.tile([P, KC, D], BF16)
            for c in range(KC):
                nc.gpsimd.dma_start(out=w1_sb[:, c, :], in_=w1_d[ts(c, P), :])
            wsl = const.tile([P, TT], F32)
            nc.gpsimd.dma_start(out=wsl, in_=ws_d.rearrange("(t p) -> p t", p=P))
            w2_sb = const.tile([P, KC, D], BF16)
            for c in range(KC):
                # separate queue from W1 so mm2(0) isn't serialized behind it
                nc.scalar.dma_start(out=w2_sb[:, c, :], in_=w2_d[ts(c, P), :])
            if affine:
                b1r = const.tile([P, D], F32)
                nc.gpsimd.dma_start(out=b1r, in_=_bcast_rows(_row1(b1_d[:])))
                g1r = const.tile([P, D], BF16)
                nc.gpsimd.dma_start(out=g1r, in_=_bcast_rows(_row1(g1_d[:])))
                be1r = const.tile([P, D], BF16)
                nc.gpsimd.dma_start(out=be1r, in_=_bcast_rows(_row1(be1_d[:])))
                b2r = const.tile([P, D], F32)
                nc.gpsimd.dma_start(out=b2r, in_=_bcast_rows(_row1(b2_d[:])))
                g2r = const.tile([P, D], BF16)
                nc.gpsimd.dma_start(out=g2r, in_=_bcast_rows(_row1(g2_d[:])))
                be2r = const.tile([P, D], BF16)
                nc.gpsimd.dma_start(out=be2r, in_=_bcast_rows(_row1(be2_d[:])))

            # x^T streamed per tile (sync DMA queue)
            xt_sb = const.tile([P, KC, C], BF16)
            for t in range(TT):
                nc.sync.dma_start(
                    out=xt_sb[:, :, ts(t, P)],
                    in_=xT_d[:, ts(t, P)].rearrange("(c p) q -> p c q", p=P),
                )

            def emit_mm1(t):
                z = zp.tile([P, D], F32, tag="z")
                for n in range(NCH):
                    for c in range(KC):
                        nc.tensor.matmul(
                            z[:, ds(n * 512, 512)],
                            xt_sb[:, c, ts(t, P)],
                            w1_sb[:, c, ds(n * 512, 512)],
                            start=(c == 0),
                            stop=(c == KC - 1),
                        )
                return z

            def ln_stats(z, tag, brt):
                """bn stats on a [P, D] PSUM tile (+optional bias add first).

                Returns (mean, rstd) as [P,1] tiles."""
                if brt is not None:
                    for n in range(NCH):
                        nc.vector.tensor_tensor(
                            out=z[:, ds(n * 512, 512)],
                            in0=z[:, ds(n * 512, 512)],
                            in1=brt[:, ds(n * 512, 512)],
                            op=ALU.add,
                        )
                st = statp.tile([P, NCH, 6], F32, tag=f"st{tag}")
                for n in range(NCH):
                    nc.vector.bn_stats(out=st[:, n, :], in_=z[:, ds(n * 512, 512)])
                mv = statp.tile([P, 2], F32, tag=f"mv{tag}")
                nc.vector.bn_aggr(out=mv, in_=st)
                sd = statp.tile([P, 1], F32, tag=f"sd{tag}")
                nc.scalar.activation(out=sd, in_=mv[:, 1:2], func=AF.Sqrt, bias=eps_sb)
                rs = statp.tile([P, 1], F32, tag=f"rs{tag}")
                nc.vector.reciprocal(out=rs, in_=sd)
                return mv, rs

            def emit_chain1(t, z):
                mv, rs = ln_stats(z, "1", b1r if affine else None)
                nm = statp.tile([P, 1], F32, tag="nm1")
                nc.vector.tensor_scalar(
                    out=nm, in0=mv[:, 0:1], scalar1=rs, scalar2=-1.0,
                    op0=ALU.mult, op1=ALU.mult,
                )
                u = up.tile([P, D], BF16, tag="u")
                if affine:
                    n1 = workp.tile([P, D], F32, tag="n1")
                    nc.scalar.activation(
                        out=n1, in_=z, func=AF.Identity, bias=nm, scale=rs
                    )
                    nc.vector.tensor_tensor(out=n1, in0=n1, in1=g1r, op=ALU.mult)
                    nc.gpsimd.tensor_tensor(out=n1, in0=n1, in1=be1r, op=ALU.add)
                    nc.scalar.activation(out=u, in_=n1, func=AF.Relu)
                else:
                    nc.scalar.activation(out=u, in_=z, func=AF.Relu, bias=nm, scale=rs)
                return u

            def emit_mm2(t, u):
                utps = utp.tile([P, D], BF16, tag="ut")
                for c in range(KC):
                    nc.tensor.transpose(utps[:, ts(c, P)], u[:, ts(c, P)], id_bf16)
                uT = uTp.tile([P, KC, P], BF16, tag="uT")
                utv = utps.rearrange("p (c q) -> p c q", c=KC)
                nc.scalar.copy(out=uT[:, 0 : KC // 2, :], in_=utv[:, 0 : KC // 2, :])
                nc.vector.tensor_copy(
                    out=uT[:, KC // 2 :, :], in_=utv[:, KC // 2 :, :]
                )
                z2 = z2p.tile([P, D], F32, tag="z2")
                for n in range(NCH):
                    for c in range(KC):
                        nc.tensor.matmul(
                            z2[:, ds(n * 512, 512)],
                            uT[:, c, :],
                            w2_sb[:, c, ds(n * 512, 512)],
                            start=(c == 0),
                            stop=(c == KC - 1),
                        )
                return z2

            def emit_chain2(t, z2):
                mv, rs = ln_stats(z2, "2", b2r if affine else None)
                rw = statp.tile([P, 1], F32, tag="rw")
                nc.vector.tensor_scalar_mul(out=rw, in0=rs, scalar1=wsl[:, t : t + 1])
                nm = statp.tile([P, 1], F32, tag="nm2")
                nc.vector.tensor_scalar(
                    out=nm, in0=mv[:, 0:1], scalar1=rw, scalar2=-1.0,
                    op0=ALU.mult, op1=ALU.mult,
                )
                y = yp.tile([P, D], F32, tag="y")
                if affine:
                    # y = w*((z2-m)*rstd*g2 + be2) ; rw = w*rstd already
                    n2 = workp.tile([P, D], F32, tag="n2")
                    nc.scalar.activation(
                        out=n2, in_=z2, func=AF.Identity, bias=nm, scale=rw
                    )
                    nc.gpsimd.tensor_tensor(out=n2, in0=n2, in1=g2r, op=ALU.mult)
                    nc.vector.scalar_tensor_tensor(
                        out=y, in0=be2r, scalar=wsl[:, t : t + 1], in1=n2,
                        op0=ALU.mult, op1=ALU.add,
                    )
                    nc.gpsimd.dma_start(out=y_d[ts(t, P), :], in_=y)
                elif t == TT - 1:
                    # last tile: chunked epilogue so the tail drains sooner
                    for n in range(NCH):
                        nc.scalar.activation(
                            out=y[:, ds(n * 512, 512)],
                            in_=z2[:, ds(n * 512, 512)],
                            func=AF.Identity, bias=nm, scale=rw,
                        )
                        nc.gpsimd.dma_start(
                            out=y_d[ts(t, P), ds(n * 512, 512)],
                            in_=y[:, ds(n * 512, 512)],
                        )
                else:
                    nc.scalar.activation(
                        out=y, in_=z2, func=AF.Identity, bias=nm, scale=rw
                    )
                    nc.gpsimd.dma_start(out=y_d[ts(t, P), :], in_=y)

            # software-pipelined emission: mm1(t) fills the PE while the LN1
            # chain of tile t-1 completes; then uT(t-1)+mm2(t-1) run.
            prev = None
            for t in range(TT):
                z = emit_mm1(t)
                u = emit_chain1(t, z)
                if prev is not None:
                    pt, pu = prev
                    z2 = emit_mm2(pt, pu)
                    emit_chain2(pt, z2)
                prev = (t, u)
            pt, pu = prev
            z2 = emit_mm2(pt, pu)
            emit_chain2(pt, z2)

    nc.compile()
    return nc


_nc_cache = {}
_nc_lock = threading.Lock()


def _get_nc(C, affine, num_devices=N_CORES):
    key = (C, affine, num_devices)
    with _nc_lock:
        if key not in _nc_cache:
            _nc_cache[key] = build_ffn_nc(C, affine, num_devices)
        return _nc_cache[key]


def _route(xf, gate_W, gate_b):
    """Host-side top-2 routing. Returns per-expert token ids + weights."""
    sc = xf.astype(np.float64) @ gate_W.astype(np.float64) + gate_b.astype(
        np.float64
    )
    top2 = np.argsort(-sc, axis=-1, kind="stable")[:, :K_TOP]
    s2 = np.take_along_axis(sc, top2, axis=-1)
    ex = np.exp(s2 - s2.max(axis=-1, keepdims=True))
    w = (ex / ex.sum(axis=-1, keepdims=True)).astype(np.float32)
    idxs, wts = [], []
    for e in range(E):
        mask = top2 == e  # [BN, 2]
        sel = mask.any(axis=-1)
        idx = np.flatnonzero(sel)
        wt = np.where(mask[idx, 0], w[idx, 0], w[idx, 1])
        idxs.append(idx)
        wts.append(wt)
    return idxs, wts


def kernel(**inputs) -> np.ndarray:
    from concourse.bass_utils import run_bass_kernel_spmd

    x = np.ascontiguousarray(np.asarray(inputs["x"], dtype=np.float32))
    B, N, Dd = x.shape
    assert Dd == D, (B, N, Dd)
    BN = B * N
    xf = x.reshape(BN, D)
    W1 = np.asarray(inputs["W1"], dtype=np.float32)
    W2 = np.asarray(inputs["W2"], dtype=np.float32)
    b1 = np.asarray(inputs["b1"], dtype=np.float32)
    g1 = np.asarray(inputs["g1"], dtype=np.float32)
    be1 = np.asarray(inputs["be1"], dtype=np.float32)
    b2 = np.asarray(inputs["b2"], dtype=np.float32)
    g2 = np.asarray(inputs["g2"], dtype=np.float32)
    be2 = np.asarray(inputs["be2"], dtype=np.float32)

    affine = not (
        np.all(b1 == 0.0)
        and np.all(be1 == 0.0)
        and np.all(b2 == 0.0)
        and np.all(be2 == 0.0)
        and np.all(g1 == 1.0)
        and np.all(g2 == 1.0)
    )

    idxs, wts = _route(
        xf,
        np.asarray(inputs["gate_W"], dtype=np.float32),
        np.asarray(inputs["gate_b"], dtype=np.float32),
    )

    halves_per_e = N_CORES // E
    C = 0
    for e in range(E):
        C = max(C, -(-((len(idxs[e]) + halves_per_e - 1) // halves_per_e) // P) * P)
    C = max(C, P)

    in_maps = []
    chunks = []  # (token-id slice, valid count) per core
    for e in range(E):
        w1e = np.ascontiguousarray(W1[e].astype(BF))
        w2e = np.ascontiguousarray(W2[e].astype(BF))
        base = {"W1": w1e, "W2": w2e}
        if affine:
            base.update(
                b1=np.ascontiguousarray(b1[e]),
                g1=np.ascontiguousarray(g1[e].astype(BF)),
                be1=np.ascontiguousarray(be1[e].astype(BF)),
                b2=np.ascontiguousarray(b2[e]),
                g2=np.ascontiguousarray(g2[e].astype(BF)),
                be2=np.ascontiguousarray(be2[e].astype(BF)),
            )
        for h in range(halves_per_e):
            idx = idxs[e][h * C : (h + 1) * C]
            wt = wts[e][h * C : (h + 1) * C]
            v = len(idx)
            xg = np.zeros((C, D), dtype=BF)
            xg[:v] = xf[idx].astype(BF)
            ws = np.zeros((C,), dtype=np.float32)
            ws[:v] = wt
            in_maps.append(
                dict(base, xT=np.ascontiguousarray(xg.T), wslot=ws)
            )
            chunks.append((idx, v))

    nc = _get_nc(C, affine, N_CORES)
    res = run_bass_kernel_spmd(nc, in_maps, core_ids=list(range(N_CORES)))

    out = xf.copy()
    for core, (idx, v) in enumerate(chunks):
        if v:
            y = np.asarray(res.results[core]["y"], dtype=np.float32)
            out[idx] += y[:v]
    return out.reshape(B, N, Dd).astype(np.float32)
